# revision 1
# baseline (speedup 1.0000x reference)
"""Trainium2 Bass kernel for ConvScoreSSREM loss.

Computes, for B=16384 rows (data-parallel, 2048 rows per NeuronCore x 8):
    cm        = contexts @ mat_M                    [B, E]
    scores_k  = sum_e cm[b,e] * res_k[b,e]          k in 0..4
    out[b]    = log_softmax(scores)[:, 0]

Per-core plan (2048 rows, E=1024):
  - mat_M resident in SBUF as fp32r chunks (full-rate PE operand), staged
    through a small fp32 buffer.
  - row-tiles processed in pairs (256 rows): 1MB DMAs, split across the two
    HWDGE rings (sync + scalar) for parallel descriptor generation.
  - per 128-row tile: PE-transpose contexts (fp32, exact) to [e,b]; 16 fp32r
    matmuls accumulate cm[128b,1024e'] in PSUM; ACT copies cm to SBUF (frees
    the PSUM bank early, avoids DVE's PSUM access latency); 5 fused DVE
    multiply+reduce (scalar_tensor_tensor) ops produce the scores.
  - one log-softmax tail over the [128, 16, 5] score tile, single DMA out.
"""

import numpy as np

import concourse.bacc as bacc
import concourse.mybir as mybir
import concourse.tile as tile
from concourse import masks
from concourse.bass_utils import run_bass_kernel_spmd

B = 16384
E = 1024
NCORES = 8
BS = B // NCORES  # 2048 rows per core
P = 128
NT = BS // P      # 16 row-tiles per core
NG = NT // 2      # 8 pair-groups
KC = E // P       # 8 contraction chunks
NK = 5            # number of res tensors
NHALF = 512       # matmul moving free-dim (one PSUM bank of fp32)

F32 = mybir.dt.float32
F32R = mybir.dt.float32r

RES_NAMES = ["res0", "res1", "res2", "res3", "res4"]


DEFAULT_OPTS = dict(
    pair=True,         # 1MB pair loads vs 0.5MB single-tile loads
    split_rings=True,  # issue loads on both HWDGE rings (sync + scalar)
    group_copies=True, # drain 4 transposes per ACT copy vs 1
    use_cms=True,      # ACT-copy cm PSUM->SBUF before the DVE score ops
                       # (frees PSUM sooner; avoids DVE PSUM access latency)
    pcm_bufs=3,
    ptr_bufs=2,
    res_bufs=12,
    ctx_bufs=3,
    cms_bufs=3,
    ctx_split=False,     # load ctx per 128-row tile (0.5MB) even when pair=True
    m_pair=False,        # load mat_M in 1MB pair chunks (4 DMAs)
    m_after_first=False, # emit group-0 input loads before the mat_M loads
    h_outer=False,       # run each 512-wide PSUM bank's matmuls to completion
                         # before the other so DVE can start on bank 0 early
    junk_bufs=2,
)


def build_nc(repeat=1, internal_inputs=False, opts=None):
    """Build + compile the single-core Bass program (same program on all 8 cores).

    repeat>1 replays the steady-state compute loop; internal_inputs=True reads
    contexts/res from internal DRAM scratch instead of ExternalInputs (both are
    timing aids only)."""
    nc = bacc.Bacc("TRN2", debug=False, enable_asserts=False, num_devices=NCORES)

    if internal_inputs:
        ctx_d = nc.dram_tensor("contexts_i", (BS, E), F32, kind="Internal")
        res_d = [nc.dram_tensor(n + "_i", (BS, E), F32, kind="Internal") for n in RES_NAMES]
    else:
        ctx_d = nc.dram_tensor("contexts", (BS, E), F32, kind="ExternalInput")
        res_d = [nc.dram_tensor(n, (BS, E), F32, kind="ExternalInput") for n in RES_NAMES]
    m_d = nc.dram_tensor("mat_M", (E, E), F32, kind="ExternalInput")
    out_d = nc.dram_tensor("out", (BS,), F32, kind="ExternalOutput")

    o = dict(DEFAULT_OPTS)
    if opts:
        o.update(opts)
    with tile.TileContext(nc) as tc:
        _body(nc, tc, ctx_d.ap(), [r.ap() for r in res_d], m_d.ap(), out_d.ap(),
              repeat=repeat, o=o)

    nc.compile()
    return nc


def _body(nc, tc, ctx_d, res_d, m_d, out_d, repeat=1, o=None):
    o = o or DEFAULT_OPTS
    na = 2 if o["pair"] else 1
    ng = NT // na
    # DRAM views with groups split out: row (g*na + a)*128 + p.
    # DMA pairs source/dest elements in flat AP order, so the DRAM view must
    # match the SBUF tile's [p, a, e] dim order.
    ctx_g = ctx_d.rearrange("(g a p) e -> g p a e", a=na, p=P)
    res_g = [r.rearrange("(g a p) e -> g p a e", a=na, p=P) for r in res_d]

    with (
        tc.tile_pool(name="mpool", bufs=1) as mpool,
        tc.tile_pool(name="mstage", bufs=2) as mstage,
        tc.tile_pool(name="ctxp", bufs=o["ctx_bufs"]) as ctxp,
        tc.tile_pool(name="resp", bufs=o["res_bufs"]) as resp,
        tc.tile_pool(name="ctxTp", bufs=2) as ctxTp,
        tc.tile_pool(name="cmsb", bufs=o["cms_bufs"]) as cmsb,
        tc.tile_pool(name="junkp", bufs=o["junk_bufs"]) as junkp,
        tc.tile_pool(name="smallp", bufs=1) as smallp,
        tc.tile_pool(name="pcm", bufs=o["pcm_bufs"], space="PSUM") as pcm,
        tc.tile_pool(name="ptr", bufs=o["ptr_bufs"], space="PSUM") as ptr,
    ):
        ident = smallp.tile([P, P], F32)
        masks.make_identity(nc, ident[:])

        # mat_M resident in fp32r: m_sbr[p, k, :] = M[k*128 + p, :]
        m_sbr = mpool.tile([P, KC, E], F32R)
        m_pair_v = m_d.rearrange("(q a p) e -> q p a e", a=2, p=P)

        def load_m():
            if o["m_pair"]:
                for q in range(KC // 2):
                    stg = mstage.tile([P, 2, E], F32, tag="mstg")
                    nc.sync.dma_start(stg[:], m_pair_v[q])
                    nc.scalar.copy(
                        m_sbr[:, 2 * q : 2 * q + 2, :].rearrange("p a e -> p (a e)"),
                        stg[:].rearrange("p a e -> p (a e)"),
                    )
            else:
                for k in range(KC):
                    stg = mstage.tile([P, E], F32, tag="mstg")
                    nc.sync.dma_start(stg[:], m_d[k * P : (k + 1) * P, :])
                    nc.scalar.copy(m_sbr[:, k, :], stg[:])

        if not o["m_after_first"]:
            load_m()

        scores = smallp.tile([P, NT, NK], F32)

        def load_group(g):
            if o["ctx_split"] and na == 2:
                ctx_t = ctxp.tile([P, na, E], F32, tag="ctx")
                for a in range(na):
                    (nc.scalar if o["split_rings"] else nc.sync).dma_start(
                        ctx_t[:, a, :], ctx_g[g, :, a, :]
                    )
            else:
                ctx_t = ctxp.tile([P, na, E], F32, tag="ctx")
                (nc.scalar if o["split_rings"] else nc.sync).dma_start(ctx_t[:], ctx_g[g])
            res_t = []
            for k in range(NK):
                r = resp.tile([P, na, E], F32, tag="res")
                eng = nc.sync if (k < 3 or not o["split_rings"]) else nc.scalar
                eng.dma_start(r[:], res_g[k][g])
                res_t.append(r)
            return ctx_t, res_t

        for _rep in range(repeat):
            for g in range(ng):
                ctx_t, res_t = load_group(g)
                if _rep == 0 and g == 0 and o["m_after_first"]:
                    load_m()

                for a in range(na):
                    t = na * g + a
                    # transpose contexts tile: ctxT[:, k, :] = ctx[:, k*128:...].T
                    ctxT = ctxTp.tile([P, KC, P], F32R, tag="ctxT")
                    if o["group_copies"]:
                        # 4 transposes land in one PSUM bank ([128,512]); one
                        # ACT copy drains each bank
                        for q in range(2):
                            pt = ptr.tile([P, 4, P], F32, tag="pt")
                            for j in range(4):
                                k = 4 * q + j
                                nc.tensor.transpose(
                                    pt[:, j, :], ctx_t[:, a, k * P : (k + 1) * P], ident[:]
                                )
                            nc.scalar.copy(ctxT[:, 4 * q : 4 * q + 4, :], pt[:])
                    else:
                        for k in range(KC):
                            pt = ptr.tile([P, P], F32, tag="pt")
                            nc.tensor.transpose(
                                pt[:], ctx_t[:, a, k * P : (k + 1) * P], ident[:]
                            )
                            nc.scalar.copy(ctxT[:, k, :], pt[:])

                    # cm[128b, 1024e'] accumulated over 8 contraction chunks
                    cm = pcm.tile([P, E], F32, tag="cm")
                    hk = (
                        [(h, k) for h in range(2) for k in range(KC)]
                        if o["h_outer"]
                        else [(h, k) for k in range(KC) for h in range(2)]
                    )
                    for h, k in hk:
                        nc.tensor.matmul(
                            cm[:, h * NHALF : (h + 1) * NHALF],
                            ctxT[:, k, :],
                            m_sbr[:, k, h * NHALF : (h + 1) * NHALF],
                            start=(k == 0),
                            stop=(k == KC - 1),
                        )

                    if o["use_cms"]:
                        # PSUM -> SBUF decouples the PSUM bank from the DVE reads
                        cm_s = cmsb.tile([P, E], F32, tag="cms")
                        nc.scalar.copy(cm_s[:], cm[:])
                    else:
                        cm_s = cm

                    # scores[:, t, k] = sum_e' cm * res_k (fused mul+accum on DVE)
                    for k in range(NK):
                        junk = junkp.tile([P, E], F32, tag="junk")
                        nc.vector.scalar_tensor_tensor(
                            out=junk[:],
                            in0=cm_s[:],
                            scalar=1.0,
                            in1=res_t[k][:, a, :],
                            op0=mybir.AluOpType.mult,
                            op1=mybir.AluOpType.mult,
                            accum_out=scores[:, t, k : k + 1],
                        )

        # ---- log-softmax tail over [P, NT, NK] ----
        mx = smallp.tile([P, NT], F32)
        nc.vector.tensor_reduce(
            out=mx[:], in_=scores[:], axis=mybir.AxisListType.X, op=mybir.AluOpType.max
        )
        d = smallp.tile([P, NT, NK], F32)
        mx_b = mx[:, :, None].broadcast_to([P, NT, NK])
        nc.vector.tensor_tensor(
            out=d[:], in0=scores[:], in1=mx_b, op=mybir.AluOpType.subtract
        )
        ex = smallp.tile([P, NT, NK], F32)
        nc.scalar.activation(ex[:], d[:], mybir.ActivationFunctionType.Exp)
        ssum = smallp.tile([P, NT], F32)
        nc.vector.tensor_reduce(
            out=ssum[:], in_=ex[:], axis=mybir.AxisListType.X, op=mybir.AluOpType.add
        )
        lse = smallp.tile([P, NT], F32)
        nc.scalar.activation(lse[:], ssum[:], mybir.ActivationFunctionType.Ln)
        outsb = smallp.tile([P, NT], F32)
        nc.vector.tensor_sub(outsb[:], d[:, :, 0], lse[:])

        nc.sync.dma_start(out_d.rearrange("(t p) -> p t", p=P), outsb[:])


_NC_CACHE = None


def _get_nc():
    global _NC_CACHE
    if _NC_CACHE is None:
        _NC_CACHE = build_nc()
    return _NC_CACHE


def make_in_maps(contexts, res_pos, res_neg1, res_neg2, res_neg3, res_neg4, mat_M):
    contexts = np.asarray(contexts, dtype=np.float32)
    ress = [
        np.asarray(r, dtype=np.float32)
        for r in (res_pos, res_neg1, res_neg2, res_neg3, res_neg4)
    ]
    mat_M = np.asarray(mat_M, dtype=np.float32)
    in_maps = []
    for c in range(NCORES):
        sl = slice(c * BS, (c + 1) * BS)
        m = {"contexts": contexts[sl], "mat_M": mat_M}
        for name, r in zip(RES_NAMES, ress):
            m[name] = r[sl]
        in_maps.append(m)
    return in_maps


def kernel(contexts, res_pos, res_neg1, res_neg2, res_neg3, res_neg4, mat_M):
    nc = _get_nc()
    in_maps = make_in_maps(
        contexts, res_pos, res_neg1, res_neg2, res_neg3, res_neg4, mat_M
    )
    res = run_bass_kernel_spmd(nc, in_maps, core_ids=list(range(NCORES)))
    out = np.concatenate([res.results[c]["out"] for c in range(NCORES)])
    return out.astype(np.float32, copy=False)



# revision 5
# speedup vs baseline: 1.0262x; 1.0262x over previous
"""Trainium2 Bass kernel for ConvScoreSSREM loss (fp16 data path).

Computes, for B=16384 rows (data-parallel, 2048 rows per NeuronCore x 8):
    cm        = contexts @ mat_M                    [B, E]
    scores_k  = sum_e cm[b,e] * res_k[b,e]          k in 0..4
    out[b]    = log_softmax(scores)[:, 0]

All inputs are cast to fp16 on the host before upload (the 2e-2 output
tolerance leaves ~40x margin at fp16), which halves HBM traffic: the
per-core steady state reads 24MB (ctx 4MB + 5 res of 4MB) against the
~356 GB/s HBM-per-NC limit => ~67us/pass DMA floor.

Per-core plan (2048 rows, E=1024):
  - mat_M resident in SBUF as fp16 [128, 8, 1024], loaded in one DMA.
  - row-tiles processed in quad groups (512 rows): 1MB DMAs split across
    the two HWDGE rings (sync + scalar).
  - per 128-row tile: 8 PE transposes of contexts (f16, 1 cyc/row) drained
    by 2 grouped ACT copies (cast f32->f16); 16 f16 matmuls (N=512)
    accumulate cm[128,1024] fp32 in PSUM; ACT copies cm to SBUF as
    cm' = cm/32 in f16 (unit variance, so f16 squares stay accurate).
  - score dots are split across engines to stay under the DMA floor:
      * (5 - n_act) dots: fused DVE scalar_tensor_tensor
        (1 elem/cycle @0.96GHz), accum fp32, scalar=32 undoes the scaling.
      * n_act dots: DVE tensor_tensor add x = cm' + res (f16 packed 2x
        mode) + ACT Square-with-accum of x and of res; with the shared
        ACT Square-accum of cm' per tile, s = 16*(sum(x^2) - sum(cm'^2)
        - sum(res^2)).  This moves ~half the dot work to the ACT engine.
  - one log-softmax tail over the [128, 16, 5] score tile, single DMA out.
"""

import numpy as np

import concourse.bacc as bacc
import concourse.mybir as mybir
import concourse.tile as tile
from concourse import masks
from concourse.bass_utils import run_bass_kernel_spmd

B = 16384
E = 1024
NCORES = 8
BS = B // NCORES  # 2048 rows per core
P = 128
NT = BS // P      # 16 row-tiles per core
KC = E // P       # 8 contraction chunks
NK = 5            # number of res tensors
NHALF = 512       # matmul moving free-dim (one PSUM bank of fp32)
CMS = 32.0        # cm scale: cm' = cm/32 has ~unit variance

F32 = mybir.dt.float32
F16 = mybir.dt.float16

RES_NAMES = ["res0", "res1", "res2", "res3", "res4"]


DEFAULT_OPTS = dict(
    na=4,           # row-tiles per load group (1MB DMAs)
    n_act=0,        # ACT square-trick dots: 0 — ACT SBUF-source ops hit the
                    # 2.3x silicon errata, so routing dots via ACT Square loses
    split_rings=True,
    res_bufs=10,
    ctx_bufs=3,
    cms_bufs=3,
    ctxT_bufs=2,
    x_bufs=2,
    junk_bufs=2,
    pcm_bufs=3,
    ptr_bufs=2,
)


def build_nc(repeat=1, internal_inputs=False, opts=None):
    """Build + compile the single-core Bass program (same program on all 8 cores).

    repeat>1 replays the steady-state compute loop; internal_inputs=True reads
    contexts/res from internal DRAM scratch instead of ExternalInputs (both are
    timing aids only)."""
    nc = bacc.Bacc("TRN2", debug=False, enable_asserts=False, num_devices=NCORES)

    if internal_inputs:
        ctx_d = nc.dram_tensor("contexts_i", (BS, E), F16, kind="Internal")
        res_d = [nc.dram_tensor(n + "_i", (BS, E), F16, kind="Internal") for n in RES_NAMES]
    else:
        ctx_d = nc.dram_tensor("contexts", (BS, E), F16, kind="ExternalInput")
        res_d = [nc.dram_tensor(n, (BS, E), F16, kind="ExternalInput") for n in RES_NAMES]
    m_d = nc.dram_tensor("mat_M", (E, E), F16, kind="ExternalInput")
    out_d = nc.dram_tensor("out", (BS,), F32, kind="ExternalOutput")

    o = dict(DEFAULT_OPTS)
    if opts:
        o.update(opts)
    with tile.TileContext(nc) as tc:
        _body(nc, tc, ctx_d.ap(), [r.ap() for r in res_d], m_d.ap(), out_d.ap(),
              repeat=repeat, o=o)

    nc.compile()
    return nc


def _body(nc, tc, ctx_d, res_d, m_d, out_d, repeat=1, o=None):
    o = o or DEFAULT_OPTS
    na = o["na"]
    ng = NT // na
    n_act = o["n_act"]
    n_stt = NK - n_act
    Sq = mybir.ActivationFunctionType.Square

    ctx_g = ctx_d.rearrange("(g a p) e -> g p a e", a=na, p=P)
    res_g = [r.rearrange("(g a p) e -> g p a e", a=na, p=P) for r in res_d]

    with (
        tc.tile_pool(name="mpool", bufs=1) as mpool,
        tc.tile_pool(name="ctxp", bufs=o["ctx_bufs"]) as ctxp,
        tc.tile_pool(name="resp", bufs=o["res_bufs"]) as resp,
        tc.tile_pool(name="ctxTp", bufs=o["ctxT_bufs"]) as ctxTp,
        tc.tile_pool(name="cmsb", bufs=o["cms_bufs"]) as cmsb,
        tc.tile_pool(name="xp", bufs=o["x_bufs"]) as xp,
        tc.tile_pool(name="junkp", bufs=o["junk_bufs"]) as junkp,
        tc.tile_pool(name="smallp", bufs=1) as smallp,
        tc.tile_pool(name="pcm", bufs=o["pcm_bufs"], space="PSUM") as pcm,
        tc.tile_pool(name="ptr", bufs=o["ptr_bufs"], space="PSUM") as ptr,
    ):
        ident = smallp.tile([P, P], F16)
        masks.make_identity(nc, ident[:])

        # mat_M resident in fp16: m_sb[p, k, :] = M[k*128 + p, :]; one 2MB DMA
        m_sb = mpool.tile([P, KC, E], F16)
        nc.sync.dma_start(m_sb[:], m_d.rearrange("(k p) e -> p k e", p=P))

        scores = smallp.tile([P, NT, NK], F32)
        # square-trick accumulators
        scm = smallp.tile([P, NT], F32)
        sxs = smallp.tile([P, NT, max(n_act, 1)], F32)
        srs = smallp.tile([P, NT, max(n_act, 1)], F32)

        def load_group(g):
            ctx_t = ctxp.tile([P, na, E], F16, tag="ctx")
            (nc.scalar if o["split_rings"] else nc.sync).dma_start(ctx_t[:], ctx_g[g])
            res_t = []
            for k in range(NK):
                r = resp.tile([P, na, E], F16, tag="res")
                eng = nc.sync if (k < 3 or not o["split_rings"]) else nc.scalar
                eng.dma_start(r[:], res_g[k][g])
                res_t.append(r)
            return ctx_t, res_t

        for _rep in range(repeat):
            for g in range(ng):
                ctx_t, res_t = load_group(g)

                for a in range(na):
                    t = na * g + a
                    # transpose contexts tile: ctxT[:, k, :] = ctx[:, k*128:...].T
                    ctxT = ctxTp.tile([P, KC, P], F16, tag="ctxT")
                    for q in range(2):
                        pt = ptr.tile([P, 4, P], F16, tag="pt")
                        for j in range(4):
                            k = 4 * q + j
                            nc.tensor.transpose(
                                pt[:, j, :], ctx_t[:, a, k * P : (k + 1) * P], ident[:]
                            )
                        nc.scalar.copy(ctxT[:, 4 * q : 4 * q + 4, :], pt[:])

                    # cm[128b, 1024e'] accumulated over 8 contraction chunks
                    cm = pcm.tile([P, E], F32, tag="cm")
                    for k in range(KC):
                        for h in range(2):
                            nc.tensor.matmul(
                                cm[:, h * NHALF : (h + 1) * NHALF],
                                ctxT[:, k, :],
                                m_sb[:, k, h * NHALF : (h + 1) * NHALF],
                                start=(k == 0),
                                stop=(k == KC - 1),
                            )

                    # cm' = cm/32 in f16 (PSUM -> SBUF, frees the PSUM bank)
                    cm_s = cmsb.tile([P, E], F16, tag="cms")
                    nc.scalar.mul(cm_s[:], cm[:], 1.0 / CMS)
                    if n_act:
                        jc = junkp.tile([P, E], F16, tag="junk")
                        nc.scalar.activation(
                            jc[:], cm_s[:], Sq, accum_out=scm[:, t : t + 1]
                        )

                    # scores[:, t, k] = sum_e' cm * res_k
                    for k in range(n_stt):
                        junk = junkp.tile([P, E], F16, tag="junk")
                        nc.vector.scalar_tensor_tensor(
                            out=junk[:],
                            in0=cm_s[:],
                            scalar=CMS,
                            in1=res_t[k][:, a, :],
                            op0=mybir.AluOpType.mult,
                            op1=mybir.AluOpType.mult,
                            accum_out=scores[:, t, k : k + 1],
                        )
                    for j in range(n_act):
                        k = n_stt + j
                        x = xp.tile([P, E], F16, tag="x")
                        nc.vector.tensor_tensor(
                            out=x[:], in0=cm_s[:], in1=res_t[k][:, a, :],
                            op=mybir.AluOpType.add,
                        )
                        jx = junkp.tile([P, E], F16, tag="junk")
                        nc.scalar.activation(
                            jx[:], x[:], Sq, accum_out=sxs[:, t, j : j + 1]
                        )
                        jr = junkp.tile([P, E], F16, tag="junk")
                        nc.scalar.activation(
                            jr[:], res_t[k][:, a, :], Sq,
                            accum_out=srs[:, t, j : j + 1],
                        )

        # ---- assemble square-trick scores:
        #      s = 16*(sum(x^2) - sum(cm'^2) - sum(res^2)) ----
        if n_act:
            d1 = smallp.tile([P, NT, n_act], F32)
            nc.vector.tensor_sub(d1[:], sxs[:, :, :n_act], srs[:, :, :n_act])
            scm_b = scm[:, :, None].broadcast_to([P, NT, n_act])
            d2 = smallp.tile([P, NT, n_act], F32)
            nc.vector.tensor_sub(d2[:], d1[:], scm_b)
            nc.vector.tensor_scalar_mul(
                scores[:, :, n_stt:], d2[:], CMS / 2.0
            )

        # ---- log-softmax tail over [P, NT, NK] ----
        mx = smallp.tile([P, NT], F32)
        nc.vector.tensor_reduce(
            out=mx[:], in_=scores[:], axis=mybir.AxisListType.X, op=mybir.AluOpType.max
        )
        d = smallp.tile([P, NT, NK], F32)
        mx_b = mx[:, :, None].broadcast_to([P, NT, NK])
        nc.vector.tensor_tensor(
            out=d[:], in0=scores[:], in1=mx_b, op=mybir.AluOpType.subtract
        )
        ex = smallp.tile([P, NT, NK], F32)
        nc.scalar.activation(ex[:], d[:], mybir.ActivationFunctionType.Exp)
        ssum = smallp.tile([P, NT], F32)
        nc.vector.tensor_reduce(
            out=ssum[:], in_=ex[:], axis=mybir.AxisListType.X, op=mybir.AluOpType.add
        )
        lse = smallp.tile([P, NT], F32)
        nc.scalar.activation(lse[:], ssum[:], mybir.ActivationFunctionType.Ln)
        outsb = smallp.tile([P, NT], F32)
        nc.vector.tensor_sub(outsb[:], d[:, :, 0], lse[:])

        nc.sync.dma_start(out_d.rearrange("(t p) -> p t", p=P), outsb[:])


_NC_CACHE = None


def _get_nc():
    global _NC_CACHE
    if _NC_CACHE is None:
        _NC_CACHE = build_nc()
    return _NC_CACHE


def make_in_maps(contexts, res_pos, res_neg1, res_neg2, res_neg3, res_neg4, mat_M):
    contexts = np.asarray(contexts, dtype=np.float16)
    ress = [
        np.asarray(r, dtype=np.float16)
        for r in (res_pos, res_neg1, res_neg2, res_neg3, res_neg4)
    ]
    mat_M = np.asarray(mat_M, dtype=np.float16)
    in_maps = []
    for c in range(NCORES):
        sl = slice(c * BS, (c + 1) * BS)
        m = {"contexts": contexts[sl], "mat_M": mat_M}
        for name, r in zip(RES_NAMES, ress):
            m[name] = r[sl]
        in_maps.append(m)
    return in_maps


def kernel(contexts, res_pos, res_neg1, res_neg2, res_neg3, res_neg4, mat_M):
    nc = _get_nc()
    in_maps = make_in_maps(
        contexts, res_pos, res_neg1, res_neg2, res_neg3, res_neg4, mat_M
    )
    res = run_bass_kernel_spmd(nc, in_maps, core_ids=list(range(NCORES)))
    out = np.concatenate([res.results[c]["out"] for c in range(NCORES)])
    return out.astype(np.float32, copy=False)
